# revision 16
# baseline (speedup 1.0000x reference)
"""Trainium2 Bass kernel for capsule-network dynamic routing (g-form).

See kernel_v2_632us.py docstring for the math.  v4 changes:
  - WG/UU columns per (og,nh) reordered to (i16, ns128) so the agreement
    i-reduction becomes 4 contiguous in-place tree-adds (VE or GpSimd)
    instead of a segmented tensor_reduce
  - phase A split per-o: even o = VE muls fused from PSUM; odd o = ACT
    evac + GpSimd muls/tree
  - blog kept in bf16
  - phase C: X-muls all on VE (GpSimd+VE concurrent broadcast muls
    contend on SBUF ports); c-transposes pipelined one og ahead so the
    PE has no og-boundary bubble
  - AllReduce payload in bf16

Layouts per core (all bf16):
  WSN [32=(nh,i), 128=ns, 1024=(o,k)] : W[nh*128+ns, i, (o,k)]
  WG  [128=(j,k), (og8,nh2,i16,ns128)]: W[nh*128+ns, i, (4og+j)*32+k]
  UTN [128=ns, (nh2,i16,b64)]         : u[b, nh*128+ns, i]
  UU2 [128=(nh,b), (i16,ns128)]       : u[b, nh*128+ns, i]
  vv  [128=(j,k), (nh2,og8,b64)]      : v[b, 4og+j, k]  (dup over nh)
"""

import numpy as np

B = 64
N_FULL = 2048
D_IN = 16
N_OUT = 32
D_OUT = 32
J = N_OUT * D_OUT  # 1024
N_CORES = 8
NL = N_FULL // N_CORES  # 256
NH = 2
NS = 128
OG = 8  # o-groups of 4

_CACHE = {}


def _pack_inputs(u, W):
    import ml_dtypes
    bf = ml_dtypes.bfloat16
    I128 = np.eye(128, dtype=np.float32).astype(bf)
    J4 = np.kron(np.eye(4, dtype=np.float32), np.ones((32, 1), np.float32)).astype(bf)
    R4 = np.ascontiguousarray(J4.T).astype(bf)
    in_maps = []
    for c in range(N_CORES):
        ul = u[:, c * NL:(c + 1) * NL, :]          # [64, 256, 16]
        Wl = W[c * NL:(c + 1) * NL]                # [256, 16, 1024]
        # WSN[ns, (t=(nh,i), j)] - single-partition-dim layout so the
        # 8 MB load goes out as 4 large DMAs instead of 32 small ones
        WSN = np.ascontiguousarray(
            Wl.reshape(NH, NS, D_IN, J).transpose(1, 0, 2, 3)
            .reshape(NS, NH * D_IN * J)).astype(bf)
        # WG[(j,k), og, nh, i, ns]
        WG = np.ascontiguousarray(
            Wl.reshape(NH, NS, D_IN, OG, 4, 32).transpose(4, 5, 3, 0, 2, 1)
            .reshape(128, OG * NH * D_IN * NS)).astype(bf)
        UTN = np.ascontiguousarray(
            ul.reshape(B, NH, NS, D_IN).transpose(2, 1, 3, 0)
            .reshape(NS, NH * D_IN * B)).astype(bf)
        # UU2[(nh,b), (i,ns)]
        UU2 = np.ascontiguousarray(
            ul.reshape(B, NH, NS, D_IN).transpose(1, 0, 3, 2)
            .reshape(NH * B, D_IN * NS)).astype(bf)
        in_maps.append({"WSN": WSN, "WG": WG, "UTN": UTN, "UU2": UU2,
                        "I128": I128, "J4": J4, "R4": R4})
    return in_maps


def _build_program():
    import concourse.bass as bass  # noqa: F401
    import concourse.tile as tile
    from concourse import bacc, mybir

    f32 = mybir.dt.float32
    bf16 = mybir.dt.bfloat16
    AF = mybir.ActivationFunctionType
    ALU = mybir.AluOpType

    nc = bacc.Bacc("TRN2", target_bir_lowering=False, debug=False,
                   num_devices=N_CORES)
    WSN_d = nc.dram_tensor("WSN", [NS, 32 * J], bf16, kind="ExternalInput").ap()
    WG_d = nc.dram_tensor("WG", [128, OG * NH * NS * D_IN], bf16,
                          kind="ExternalInput").ap()
    UTN_d = nc.dram_tensor("UTN", [NS, NH * D_IN * B], bf16,
                           kind="ExternalInput").ap()
    UU2_d = nc.dram_tensor("UU2", [NH * B, NS * D_IN], bf16,
                           kind="ExternalInput").ap()
    I128_d = nc.dram_tensor("I128", [128, 128], bf16, kind="ExternalInput").ap()
    J4_d = nc.dram_tensor("J4", [128, 4], bf16, kind="ExternalInput").ap()
    R4_d = nc.dram_tensor("R4", [4, 128], bf16, kind="ExternalInput").ap()
    v_d = nc.dram_tensor("v_out", [128, 512], f32, kind="ExternalOutput").ap()

    with tile.TileContext(nc) as tc:
        with (
            tc.tile_pool(name="wpool", bufs=1) as wpool,
            tc.tile_pool(name="state", bufs=1) as state,
            tc.tile_pool(name="scratch", bufs=2) as scratch,
            tc.tile_pool(name="smalls", bufs=2) as smalls,
            tc.tile_pool(name="psum", bufs=2, space="PSUM") as pp,
            tc.tile_pool(name="dram", bufs=2, space="DRAM") as dram,
        ):
            # ---- small inputs + WSN (pass A consumes tiles as they land) --
            utn = state.tile([NS, NH * D_IN * B], bf16, tag="utn")
            nc.sync.dma_start(utn[:], UTN_d[:])
            uu2 = state.tile([NH * B, NS * D_IN], bf16, tag="uu2")
            nc.sync.dma_start(uu2[:], UU2_d[:])
            i128 = state.tile([128, 128], bf16, tag="i128")
            nc.sync.dma_start(i128[:], I128_d[:])
            j4 = state.tile([128, 4], bf16, tag="j4")
            nc.sync.dma_start(j4[:], J4_d[:])
            r4 = state.tile([4, 128], bf16, tag="r4")
            nc.sync.dma_start(r4[:], R4_d[:])
            wsn_all = wpool.tile([NS, 32 * J], bf16, tag="wsn")
            for c in range(4):
                sl = slice(c * 8 * J, (c + 1) * 8 * J)
                nc.sync.dma_start(wsn_all[:, sl], WSN_d[:, sl])

            def wsn(t, lo, hi):
                return wsn_all[:, t * J + lo:t * J + hi]

            blog = state.tile([128, N_OUT * NS], bf16, tag="blog")
            nc.gpsimd.memset(blog[:], 0.0)
            vv = state.tile([128, NH * OG * B], bf16, tag="vv")

            # ---- pass A: sT1 = (1/32)*sum_n uhat, interleaved with DMA ----
            psA = [pp.tile([128, 512], f32, tag="stj", bufs=4,
                           name=f"psA{tj}") for tj in range(4)]
            for q in range(2):  # og = q*4 + tj
                for nh in range(NH):
                    for i in range(D_IN):
                        for tj in range(4):
                            og = q * 4 + tj
                            nc.tensor.matmul(
                                psA[tj][:, q * 256 + tj * B:
                                        q * 256 + (tj + 1) * B],
                                lhsT=wsn(nh * D_IN + i, og * 128,
                                         (og + 1) * 128),
                                rhs=utn[:, (nh * D_IN + i) * B:
                                        (nh * D_IN + i + 1) * B],
                                start=(nh == 0 and i == 0),
                                stop=(nh == 1 and i == D_IN - 1))

            # WG deferred: first needed after AR1
            wg = state.tile([128, OG * NH * NS * D_IN], bf16, tag="wg")
            for og in range(OG):
                sl = slice(og * NH * NS * D_IN, (og + 1) * NH * NS * D_IN)
                nc.sync.dma_start(wg[:, sl], WG_d[:, sl])

            def ar_squash(st_parts, scale, last):
                if last:
                    s_f = scratch.tile([128, 512], f32, tag="s_f", bufs=1)
                    for (t, psl, csl, oc) in st_parts:
                        nc.scalar.mul(s_f[psl, oc:oc + (csl.stop - csl.start)],
                                      t[psl, csl], scale)
                    nc.sync.dma_start(v_d[:], s_f[:])
                    return
                s_sb = scratch.tile([128, 512], bf16, tag="s_sb", bufs=1)
                for (t, psl, csl, oc) in st_parts:
                    nc.scalar.mul(s_sb[psl, oc:oc + (csl.stop - csl.start)],
                                  t[psl, csl], scale)
                bin_ = dram.tile([128, 512], bf16, tag="bounce_in")
                bout = dram.tile([128, 512], bf16, tag="bounce_out")
                nc.sync.dma_start(bin_[:], s_sb[:])
                nc.gpsimd.collective_compute(
                    "AllReduce", ALU.add,
                    replica_groups=[list(range(N_CORES))],
                    ins=[bin_.opt()], outs=[bout.opt()],
                )
                sTr = scratch.tile([128, 512], bf16, tag="sTr", bufs=1)
                nc.sync.dma_start(sTr[:], bout[:])
                sq = scratch.tile([128, 512], bf16, tag="sqq", bufs=1)
                nc.vector.tensor_mul(sq[:], sTr[:], sTr[:])
                n2 = pp.tile([4, 512], f32, tag="stj", bufs=4)
                nc.tensor.matmul(n2[:], lhsT=j4[:], rhs=sq[:],
                                 start=True, stop=True)
                tt = smalls.tile([4, 512], f32, tag="tt", bufs=1)
                nc.scalar.activation(tt[:], n2[:], AF.Sqrt)
                dd = smalls.tile([4, 512], f32, tag="dd", bufs=1)
                nc.scalar.add(dd[:], n2[:], 1.0)
                rr_ = smalls.tile([4, 512], f32, tag="rr", bufs=1)
                nc.vector.reciprocal(rr_[:], dd[:])
                scl = smalls.tile([4, 512], bf16, tag="scl", bufs=1)
                nc.vector.tensor_mul(scl[:], tt[:], rr_[:])
                rep = pp.tile([128, 512], f32, tag="stj", bufs=4)
                nc.tensor.matmul(rep[:], lhsT=r4[:], rhs=scl[:],
                                 start=True, stop=True)
                for nh in range(NH):
                    nc.vector.tensor_mul(
                        vv[:, nh * 512:(nh + 1) * 512], sTr[:], rep[:])

            passA_parts = []
            for og in range(OG):
                q, tj = divmod(og, 4)
                passA_parts.append(
                    (psA[tj], slice(0, 128),
                     slice(q * 256 + tj * B, q * 256 + (tj + 1) * B), og * B))
            ar_squash(passA_parts, 1.0 / N_OUT, last=False)

            # ---- routing iterations ----
            for r in (1, 2):
                # phase A: g + agreement -> blog
                # even o: VE muls fused from PSUM + VE tree
                # odd o:  ACT evac + GpSimd muls + GpSimd tree
                for o in range(N_OUT):
                    og, jj = divmod(o, 4)
                    th = scratch.tile([128, 2048], bf16, tag="th")
                    for h in range(2):  # i-halves: i in [8h, 8h+8)
                        g = pp.tile([128, 1024], f32, tag="g", bufs=2)
                        for cc in range(2):
                            for nh in range(NH):
                                lhs = vv[32 * jj:32 * jj + 32,
                                         nh * 512 + og * B:
                                         nh * 512 + (og + 1) * B]
                                base = (og * NH * NS * D_IN
                                        + nh * NS * D_IN + h * 1024 + cc * 512)
                                nc.tensor.matmul(
                                    g[B * nh:B * (nh + 1),
                                      cc * 512:(cc + 1) * 512],
                                    lhsT=lhs,
                                    rhs=wg[32 * jj:32 * jj + 32,
                                           base:base + 512],
                                    start=True, stop=True,
                                    tile_position=(32 * jj, B * nh))
                        # ACT evacuates; VE multiplies at 2x (GpSimd is
                        # kept idle: concurrent VE+GpSimd SBUF streaming
                        # degrades both ~3x)
                        ge = scratch.tile([128, 1024], bf16, tag="ge",
                                          bufs=3)
                        nc.scalar.mul(ge[:], g[:], 1.0)
                        nc.vector.tensor_mul(
                            th[:, h * 1024:(h + 1) * 1024],
                            uu2[:, h * 1024:(h + 1) * 1024], ge[:])
                    # tree-reduce over i (contiguous halves), in place;
                    # small tail stages go to GpSimd (low duty - no
                    # SBUF-port contention at this rate)
                    w_ = 1024
                    while w_ >= NS:
                        teng = nc.vector if w_ > 256 else nc.gpsimd
                        teng.tensor_add(th[:, 0:w_], th[:, 0:w_],
                                        th[:, w_:2 * w_])
                        w_ //= 2
                    nc.gpsimd.tensor_add(
                        blog[:, o * NS:(o + 1) * NS],
                        blog[:, o * NS:(o + 1) * NS], th[:, 0:NS])

                # phase B: softmax over o (no max-sub: |logits| < ~1)
                ee = state.tile([128, N_OUT * NS], bf16, tag="ee")
                nc.scalar.activation(ee[:], blog[:], AF.Exp)
                tr = scratch.tile([128, 2048], bf16, tag="tree", bufs=1)
                nc.vector.tensor_add(tr[:], ee[:, 0:2048], ee[:, 2048:4096])
                w_ = 1024
                while w_ >= NS:
                    nc.vector.tensor_add(tr[:, 0:w_], tr[:, 0:w_],
                                         tr[:, w_:2 * w_])
                    w_ //= 2
                rc = smalls.tile([128, NS], bf16, tag="rc")
                with nc.allow_low_precision(reason="softmax denom, tol 2e-2"):
                    nc.vector.reciprocal(rc[:], tr[:, 0:NS])
                nc.vector.tensor_mul(
                    ee[:].rearrange("p (o ns) -> p o ns", ns=NS),
                    ee[:].rearrange("p (o ns) -> p o ns", ns=NS),
                    rc[:].unsqueeze(1).broadcast_to([128, N_OUT, NS]))

                # phase C: transpose c, X = cT*u (VE only), sT-MMs
                # (col-tiled over jj; og+1 transposes pipelined early)
                last = (r == 2)
                stj = [pp.tile([128, 512], f32, tag="stj", bufs=4,
                               name=f"stj{r}_{jj}")
                       for jj in range(4)]

                ct_ps = {}
                ct = {}

                def emit_transposes(g_):
                    ct_ps[g_] = pp.tile([128, 512], bf16, tag="g", bufs=2,
                                        name=f"ctps{r}_{g_}")
                    for j_ in range(4):
                        o_ = g_ * 4 + j_
                        nc.tensor.transpose(
                            ct_ps[g_][:, j_ * 128:(j_ + 1) * 128],
                            ee[:, o_ * NS:(o_ + 1) * NS], i128[:])
                    ct[g_] = scratch.tile([128, 512], bf16, tag="ct",
                                          name=f"ct{r}_{g_}")
                    nc.scalar.mul(ct[g_][:], ct_ps[g_][:], 1.0)

                emit_transposes(0)
                for og in range(OG):
                    for nh in range(NH):
                        xts = []
                        for jj in range(4):
                            xt = scratch.tile([128, D_IN * B], bf16,
                                              tag="xt", bufs=10)
                            nc.vector.tensor_mul(
                                xt[:].rearrange("p (i b) -> p i b", b=B),
                                utn[:, nh * D_IN * B:(nh + 1) * D_IN * B]
                                .rearrange("p (i b) -> p i b", b=B),
                                ct[og][:, jj * 128 + nh * B:
                                        jj * 128 + (nh + 1) * B]
                                .unsqueeze(1).broadcast_to([128, D_IN, B]))
                            xts.append(xt)
                        if nh == 0 and og + 1 < OG:
                            emit_transposes(og + 1)
                        for i in range(D_IN):
                            for jj in range(4):
                                o = og * 4 + jj
                                nc.tensor.matmul(
                                    stj[jj][32 * jj:32 * jj + 32,
                                            og * B:(og + 1) * B],
                                    lhsT=wsn(nh * D_IN + i, o * 32,
                                             (o + 1) * 32),
                                    rhs=xts[jj][:, i * B:(i + 1) * B],
                                    start=(nh == 0 and i == 0),
                                    stop=(nh == 1 and i == D_IN - 1),
                                    tile_position=(0, 32 * jj))
                ar_squash(
                    [(stj[jj], slice(32 * jj, 32 * jj + 32),
                      slice(0, 512), 0) for jj in range(4)],
                    1.0, last=last)

    nc.compile()
    return nc


def _get_program():
    if "nc" not in _CACHE:
        _CACHE["nc"] = _build_program()
    return _CACHE["nc"]


def _squash_np(s, axis=-1):
    n2 = np.sum(s * s, axis=axis, keepdims=True)
    return s * (n2 / (1.0 + n2) / np.sqrt(n2))


def kernel(u, W):
    from concourse.bass_utils import run_bass_kernel_spmd

    nc = _get_program()
    in_maps = _pack_inputs(np.asarray(u, np.float32), np.asarray(W, np.float32))
    res = run_bass_kernel_spmd(nc, in_maps, list(range(N_CORES)))
    sT = np.zeros((128, 512), np.float64)
    for rm in res.results:
        sT += rm["v_out"].astype(np.float64)
    # sT[(j,k), (og,b)] = s3[b, 4*og+j, k]
    s = sT.reshape(4, 32, OG, B).transpose(3, 2, 0, 1).reshape(B, N_OUT, D_OUT)
    return _squash_np(s.astype(np.float32))
